# revision 64
# baseline (speedup 1.0000x reference)
"""Self-contained Trainium2 Bass kernel for MultiHeadAttention (v2).

Problem: B=2, S=2048, D=1024, H=16, hd=64, with the reference's
masked_fill(mask==0, -1e-09) quirk: masked scores become ~0.0, so
exp(masked) == 1.0 in fp32 and every key position participates in the
softmax denominator. Fully-masked key blocks contribute a
block-constant suffix sum of V rows (fs), added via rank-1 matmuls.

Sharding: 8 cores = 2 batches x 4 head-groups (4 heads per core).
Each core computes a partial [S, D] output (its 4 heads pushed through
the O-projection); the host sums the 4 partials per batch and adds bo.

v2 scheduling vs v1:
 - host pre-shuffles all DRAM tensors into SBUF layout so every load is
   a large simple descriptor; ~6 prioritized descriptors per DGE queue.
 - projection/outproj work woven between attention steps as fillers so
   the PE never idles while ACT runs exp (PE p-state stays high).
 - diag-first kj order per chunk; last attnU of each chunk is non-diag.
 - finalize: aups -> SBUF copy (frees PSUM fast), DVE reciprocal +
   GpSimd partition_broadcast instead of ACT Ln/Exp + DRAM round-trip.
 - folded suffixes on GpSimd (15 four-head adds), off the DVE.
 - per-st [128, 1024] output tiles, one DMA per st (16 total).
"""

import numpy as np
import ml_dtypes

import concourse.bass as bass
import concourse.bacc as bacc
import concourse.tile as tile
import concourse.mybir as mybir
from concourse.bass_utils import run_bass_kernel_spmd

BF16 = mybir.dt.bfloat16
F32 = mybir.dt.float32
NPBF16 = ml_dtypes.bfloat16
AF = mybir.ActivationFunctionType

B = 2
S = 2048
D = 1024
H = 16
HD = 64
NCORES = 8
HPC = 4            # heads per core
NPAIRS = 2         # head pairs per core
NQ = S // 128      # 16 query/key blocks of 128
QCH = 512          # query chunk width
NCH = S // QCH     # 4 chunks
KT = D // 128      # 8 contraction tiles for projections


def _emit(tc: tile.TileContext, io: dict):
    nc = tc.nc

    persist = tc.alloc_tile_pool(name="persist", bufs=1)

    # ---- constants (gpsimd; must precede first use) ----
    ones128 = persist.tile([128, 128], BF16, name="ones128")
    nc.gpsimd.memset(ones128, 1.0)

    # ---- persistent SBUF arrays ----
    qt = persist.tile([128, NPAIRS, S], BF16, name="qt")
    ktz = persist.tile([128, HPC, S], BF16, name="ktz")
    v2 = persist.tile([128, HPC, NQ, 65], BF16, name="v2")
    fs = persist.tile([128, HPC, NQ, 65], BF16, name="fs")
    att = persist.tile([128, NPAIRS, S], BF16, name="att")

    qts = persist.tile([128, KT, S], BF16, name="qts")
    kts = persist.tile([128, KT, S], BF16, name="kts")
    vts = persist.tile([128, KT, S], BF16, name="vts")
    wq = persist.tile([128, KT, 256], BF16, name="wq")
    wk = persist.tile([128, KT, 256], BF16, name="wk")
    wv = persist.tile([128, KT, 256], BF16, name="wv")
    wo = persist.tile([128, NPAIRS, D], BF16, name="wo")
    bqkv = persist.tile([1, 3, 256], BF16, name="bqkv")
    bqkt = persist.tile([128, 2, NPAIRS], F32, name="bqkt")
    bvb = persist.tile([128, 256], BF16, name="bvb")

    # ---- input DMA: few big descriptors, priority-ordered per queue ----
    # HWDGE queues: sync (SP) + scalar (Activation); gpsimd uses SWDGE.
    # Ordering matches the schedule's consumption: K+wk first (kproj(0)
    # opens the kernel), V chunk0 by ~20us, Q/K chunk1 by ~32us (qp1/kp1),
    # the rest of V by ~45us (fs suffix chain), Q/K chunks 2-3 by ~55us.
    sy, gp, sc = nc.sync, nc.gpsimd, nc.scalar
    # P0 (kq-proj inputs) + ALL of V balanced at 2.5MB per queue: the fs
    # suffix chain (-> finalize(0)) is the startup critical path and needs
    # v2 complete; vts lands ~38us this way.
    sc.dma_start(bqkv[:, :, :], io["BQKV"][:, :, :])
    sc.dma_start(bqkt[:, :, :], io["BQKT"][:, :, :])
    sy.dma_start(wk[:, :, :], io["WK"][:, :, :])
    gp.dma_start(wq[:, :, :], io["WQ"][:, :, :])
    sy.dma_start(kts[:, 0:4, 0:QCH], io["KTS"][:, 0:4, 0:QCH])
    sc.dma_start(kts[:, 4:8, 0:QCH], io["KTS"][:, 4:8, 0:QCH])
    gp.dma_start(qts[:, 0:4, 0:QCH], io["QTS"][:, 0:4, 0:QCH])
    sc.dma_start(qts[:, 4:8, 0:QCH], io["QTS"][:, 4:8, 0:QCH])
    # ktz zero halves + v2 ones column + fs seed (gpsimd, between its DMAs)
    for h in range(HPC):
        half = slice(64, 128) if h % 2 == 0 else slice(0, 64)
        nc.gpsimd.memset(ktz[half, h, :], 0.0)
    nc.gpsimd.memset(v2[:, :, :, 64:65], 1.0)
    nc.gpsimd.memset(fs[:, :, NQ - 1, :], 0.0)
    nc.gpsimd.partition_broadcast(bvb[:, :], bqkv[0:1, 2, :], channels=128)
    sy.dma_start(vts[:, 0:4, 0:QCH], io["VTS"][:, 0:4, 0:QCH])
    gp.dma_start(vts[:, 4:8, 0:QCH], io["VTS"][:, 4:8, 0:QCH])
    sc.dma_start(wv[:, :, :], io["WV"][:, :, :])
    # V-rest DESCENDING (vp15..vp4 drain order: the fs suffix chain runs
    # descending, so its adds pipeline right behind the vproj pipeline)
    sy.dma_start(vts[:, 0:4, 3 * QCH:], io["VTS"][:, 0:4, 3 * QCH:])
    gp.dma_start(vts[:, 4:8, 3 * QCH:], io["VTS"][:, 4:8, 3 * QCH:])
    sy.dma_start(vts[:, 0:4, 2 * QCH:3 * QCH], io["VTS"][:, 0:4, 2 * QCH:3 * QCH])
    gp.dma_start(vts[:, 4:8, 2 * QCH:3 * QCH], io["VTS"][:, 4:8, 2 * QCH:3 * QCH])
    sy.dma_start(vts[:, 0:4, QCH:2 * QCH], io["VTS"][:, 0:4, QCH:2 * QCH])
    gp.dma_start(vts[:, 4:8, QCH:2 * QCH], io["VTS"][:, 4:8, QCH:2 * QCH])
    # Q/K chunk1 (qp1 at end of chunk0, kp1 early chunk1), wo
    sc.dma_start(wo[:, :, :], io["WO"][:, :, :])
    sy.dma_start(qts[:, :, QCH:2 * QCH], io["QTS"][:, :, QCH:2 * QCH])
    gp.dma_start(kts[:, :, QCH:2 * QCH], io["KTS"][:, :, QCH:2 * QCH])
    # Q/K chunks 2+3
    sy.dma_start(qts[:, :, 2 * QCH:3 * QCH], io["QTS"][:, :, 2 * QCH:3 * QCH])
    gp.dma_start(kts[:, :, 2 * QCH:3 * QCH], io["KTS"][:, :, 2 * QCH:3 * QCH])
    sy.dma_start(qts[:, :, 3 * QCH:], io["QTS"][:, :, 3 * QCH:])
    gp.dma_start(kts[:, :, 3 * QCH:], io["KTS"][:, :, 3 * QCH:])

    # ---- pools ----
    pb_s = tc.alloc_tile_pool(name="pb_s", bufs=2, space="PSUM")   # 4 banks
    pb_a = tc.alloc_tile_pool(name="pb_a", bufs=2, space="PSUM")   # 4 banks
    pb_e = tc.alloc_tile_pool(name="pb_e", bufs=10)
    pb_r = tc.alloc_tile_pool(name="pb_r", bufs=2)
    pb_o = tc.alloc_tile_pool(name="pb_o", bufs=4)

    # ---- work units ----
    def qproj_unit(c, p):
        sq = slice(c * QCH, (c + 1) * QCH)
        ps = pb_s.tile([128, 2, QCH], F32, tag="sps", name=f"psq{c}_{p}")
        for t in range(KT):
            nc.tensor.matmul(ps[:, 0, :], wq[:, t, p * 128:(p + 1) * 128],
                             qts[:, t, sq], start=(t == 0), stop=(t == KT - 1))
        nc.vector.tensor_add(qt[:, p, sq], ps[:, 0, :],
                             bqkt[:, 0, p].broadcast_to([128, QCH]))

    def kproj_unit(c, p):
        sq = slice(c * QCH, (c + 1) * QCH)
        ps = pb_s.tile([128, 2, QCH], F32, tag="sps", name=f"psk{c}_{p}")
        for t in range(KT):
            nc.tensor.matmul(ps[:, 0, :], wk[:, t, p * 128:(p + 1) * 128],
                             kts[:, t, sq], start=(t == 0), stop=(t == KT - 1))
        nc.vector.tensor_add(ktz[0:64, 2 * p, sq], ps[0:64, 0, :],
                             bqkt[0:64, 1, p].broadcast_to([64, QCH]))
        nc.vector.tensor_add(ktz[64:128, 2 * p + 1, sq], ps[64:128, 0, :],
                             bqkt[64:128, 1, p].broadcast_to([64, QCH]))

    def vproj_unit(st):
        ps = pb_s.tile([128, 2, QCH], F32, tag="sps", name=f"psv{st}")
        for t in range(KT):
            nc.tensor.matmul(ps[:, 0, 0:256], vts[:, t, st * 128:(st + 1) * 128],
                             wv[:, t, :], start=(t == 0), stop=(t == KT - 1))
        nc.vector.tensor_add(v2[:, :, st, 0:64], ps[:, 0, 0:256], bvb)

    def fs_add(q):
        # fs[:, :, q] = fs[:, :, q+1] + v2[:, :, q+1]   (all 4 heads)
        nc.vector.tensor_add(fs[:, :, q, :], fs[:, :, q + 1, :],
                             v2[:, :, q + 1, :])

    ob_tiles = {}

    def outproj_unit(st, dc):
        ps = pb_s.tile([128, 2, QCH], F32, tag="sps", name=f"pso{st}_{dc}")
        for p in range(NPAIRS):
            nc.tensor.matmul(ps[:, 0, :], att[:, p, st * 128:(st + 1) * 128],
                             wo[:, p, dc * QCH:(dc + 1) * QCH],
                             start=(p == 0), stop=(p == NPAIRS - 1))
        if dc == 0:
            ob_tiles[st] = pb_o.tile([128, D], BF16, tag="ob", name=f"ob{st}")
        ob = ob_tiles[st]
        nc.vector.tensor_copy(ob[:, dc * QCH:(dc + 1) * QCH], ps[:, 0, :])
        if st >= 12:  # tail: half-size DMAs, each half leaves ASAP
            eng = (sy, gp, sc)[(2 * st + dc) % 3]
            eng.dma_start(io["OUT"][:, st, dc * QCH:(dc + 1) * QCH],
                          ob[:, dc * QCH:(dc + 1) * QCH])
        elif dc == 1:
            # sync only: scalar/gpsimd issues would stall ACT exps and
            # affine_selects mid-kernel
            sy.dma_start(io["OUT"][:, st, :], ob)

    # ---- attention ----
    aups_tiles = {}

    def attn_score(c, p, kj):
        c0 = max(kj - 4 * c, 0) * 128
        sps = pb_s.tile([128, 2, QCH], F32, tag="sps", name=f"sps{c}_{p}_{kj}")
        for hl in range(2):
            nc.tensor.matmul(
                sps[:, hl, c0:QCH],
                ktz[:, 2 * p + hl, kj * 128:(kj + 1) * 128],
                qt[:, p, c * QCH + c0:(c + 1) * QCH],
                start=True, stop=True)
        ext = pb_e.tile([128, 2, QCH], BF16, tag="ext", name=f"ext{c}_{p}_{kj}")
        nc.scalar.activation(ext[:, :, c0:QCH], sps[:, :, c0:QCH],
                             AF.Exp, scale=0.125)
        if kj >= 4 * c:  # diagonal block: masked exp entries -> 1.0
            for hl in range(2):
                nc.gpsimd.affine_select(
                    out=ext[:, hl, c0:c0 + 128],
                    in_=ext[:, hl, c0:c0 + 128],
                    compare_op=mybir.AluOpType.is_ge,
                    fill=1.0, base=0,
                    pattern=[[1, 128]], channel_multiplier=-1)
        return ext

    def attn_acc(c, p, kj, ext, first):
        c0 = max(kj - 4 * c, 0) * 128
        aups = aups_tiles[(c, p)]
        for hl in range(2):
            nc.tensor.matmul(aups[:, hl, c0:QCH],
                             v2[:, 2 * p + hl, kj, :],
                             ext[:, hl, c0:QCH],
                             start=first, stop=False)

    def finalize(c, per_st=False):
        ch = slice(c * QCH, (c + 1) * QCH)
        for p in range(NPAIRS):
            aups = aups_tiles[(c, p)]
            for hl in range(2):
                qls = [ql for ql in range(4) if 4 * c + ql < NQ - 1]
                for ql in qls:
                    nc.tensor.matmul(
                        aups[:, hl, ql * 128:(ql + 1) * 128],
                        fs[:, 2 * p + hl, 4 * c + ql, :], ones128,
                        start=False, stop=(ql == qls[-1]))
        # per-pair reciprocal chains, reading aups (PSUM) directly:
        # denom row -> partition 0 (DVE shifted copy), exp(-ln(d)) on ACT,
        # broadcast on GpSimd, normalize on DVE
        d0s, reps = [], []
        for p in range(NPAIRS):
            d0 = pb_r.tile([1, 2, QCH], F32, tag="d0", name=f"d0{c}_{p}")
            nc.vector.tensor_copy(d0[0:1, :, :],
                                  aups_tiles[(c, p)][64:65, :, :])
            d0s.append(d0)
        for p in range(NPAIRS):
            d0 = d0s[p]
            nc.scalar.activation(d0[0:1, :, :], d0[0:1, :, :], AF.Ln)
            nc.scalar.activation(d0[0:1, :, :], d0[0:1, :, :], AF.Exp,
                                 scale=-1.0)
            rep = pb_r.tile([64, 2, QCH], F32, tag="rep", name=f"rep{c}_{p}")
            nc.gpsimd.partition_broadcast(rep[0:64, :, :], d0[0:1, :, :],
                                          channels=64)
            reps.append(rep)
        if per_st:  # tail chunk: st-granular muls so outproj starts early
            for ql in range(4):
                qs = slice(ql * 128, (ql + 1) * 128)
                for p in range(NPAIRS):
                    for hl in range(2):
                        nc.vector.tensor_mul(
                            att[hl * 64:(hl + 1) * 64, p,
                                c * QCH + ql * 128:c * QCH + (ql + 1) * 128],
                            aups_tiles[(c, p)][0:64, hl, qs],
                            reps[p][0:64, hl, qs])
        else:
            for p in range(NPAIRS):
                for hl in range(2):
                    nc.vector.tensor_mul(
                        att[hl * 64:(hl + 1) * 64, p, ch],
                        aups_tiles[(c, p)][0:64, hl, :],
                        reps[p][0:64, hl, :])

    # ---- schedule ----
    kproj_unit(0, 0)
    qproj_unit(0, 0)
    kproj_unit(0, 1)
    qproj_unit(0, 1)

    def chunk(c, kj_order, fillers, per_step):
        """Attention steps with the attnU of step i emitted during step i+1
        (1-step software pipeline: PE never sits on the exp latency)."""
        for p in range(NPAIRS):
            aups_tiles[(c, p)] = pb_a.tile([65, 2, QCH], F32, tag="aups",
                                           name=f"aups{c}_{p}")
        prev = None
        for i, kj in enumerate(kj_order):
            exts = [attn_score(c, p, kj) for p in range(NPAIRS)]
            if prev is not None:
                pkj, pexts, pfirst = prev
                for p in range(NPAIRS):
                    attn_acc(c, p, pkj, pexts[p], first=pfirst)
            prev = (kj, exts, i == 0)
            n = per_step[i] if i < len(per_step) else 0
            for _ in range(n):
                if fillers:
                    fillers.pop(0)()
        pkj, pexts, pfirst = prev
        for p in range(NPAIRS):
            attn_acc(c, p, pkj, pexts[p], first=pfirst)
        while fillers:
            fillers.pop(0)()

    def fs_all():
        for q in range(NQ - 2, -1, -1):
            fs_add(q)

    # chunk 0 fillers: all remaining vproj (V is DMA-prioritized), the fs
    # chain behind them, then qp1 as its DMA lands; finalize(0) right after
    # chunk 0: weave only vp0..3 (needed by the accs); everything else
    # (vp4..15 DMA-paced, fs chain, qp1) drains AFTER the last acc so
    # finalize(0) is reached as soon as the fs data allows.
    fl0 = [lambda st=st: vproj_unit(st) for st in range(4)]
    fl0 += [lambda: vproj_unit(NQ - 1)]
    for st in range(NQ - 2, 3, -1):
        def unit(st=st):
            vproj_unit(st)
            fs_add(st)
        fl0.append(unit)
    fl0 += [lambda q=q: fs_add(q) for q in (3, 2, 1, 0)]
    chunk(0, [0, 1, 2, 3], fl0, [1, 1, 1, 1])
    finalize(0)
    # qp1 during finalize(0)'s ACT/GpSimd chain (PE is free there)
    qproj_unit(1, 0)
    qproj_unit(1, 1)

    # chunk 1: kp1 on the non-diag warmup steps, qp2 late
    fl1 = [lambda: kproj_unit(1, 0), lambda: kproj_unit(1, 1)]
    chunk(1, [0, 1, 2, 3, 4, 5, 6, 7], fl1, [1, 1, 0, 0, 0, 0, 0, 0])
    finalize(1)
    # qk2 + outproj(0) densely at the boundary, overlapping fin1's chain
    qproj_unit(2, 0)
    qproj_unit(2, 1)
    kproj_unit(2, 0)
    kproj_unit(2, 1)
    for st in range(0, 4):
        for dc in range(2):
            outproj_unit(st, dc)

    # chunk 2: clean steps (kp2 ran at the end of chunk 1)
    chunk(2, [0, 1, 8, 9, 10, 11, 2, 3, 4, 5, 6, 7], [], [0] * 12)
    finalize(2)
    qproj_unit(3, 0)
    qproj_unit(3, 1)
    kproj_unit(3, 0)
    kproj_unit(3, 1)

    # chunk 3: five score-steps pre-run (deep ext pool), then the dense
    # outproj(1)+outproj(2) block executes on PE while ACT drains their
    # exps; remaining steps are ACT-bound with a warm pipeline.
    fl3 = [lambda st=st, dc=dc: outproj_unit(st, dc)
           for st in range(4, 12) for dc in range(2)]
    chunk(3, [0, 1, 12, 13, 14, 15, 2, 3, 4, 5, 6, 7, 8, 9, 10, 11], fl3,
          [0, 0, 0, 0, 16, 0, 0, 0, 0, 0, 0, 0, 0, 0, 0, 0])
    finalize(3, per_st=True)
    for st in range(12, 16):
        for dc in range(2):
            outproj_unit(st, dc)

    pb_o.release()
    pb_r.release()
    pb_e.release()
    pb_a.release()
    pb_s.release()
    persist.release()


_CACHED = None


def _patch_act_tables():
    """Make Exp and Ln resolve to the single combined table set so the
    per-chunk recip (Ln/Exp) doesn't thrash ACT_TABLE_LOADs against the
    softmax Exp calls. Set positions (= act_func_set_id) are preserved;
    only membership of Exp/Ln in other sets is hidden from the selector."""
    from concourse import hw_specs
    orig = hw_specs.get_activation_tables

    def patched(arch):
        t = dict(orig(arch))
        if "natural_log_exp_and_others" in t:
            for name in t:
                if name != "natural_log_exp_and_others":
                    t[name] = t[name] - {AF.Exp, AF.Ln}
        return t

    bacc.get_activation_tables = patched


def _build():
    global _CACHED
    if _CACHED is not None:
        return _CACHED
    _patch_act_tables()
    nc = bacc.Bacc("TRN2", target_bir_lowering=False, debug=False)
    io = {
        "QTS": nc.dram_tensor("QTS", [128, KT, S], BF16, kind="ExternalInput").ap(),
        "KTS": nc.dram_tensor("KTS", [128, KT, S], BF16, kind="ExternalInput").ap(),
        "VTS": nc.dram_tensor("VTS", [128, KT, S], BF16, kind="ExternalInput").ap(),
        "WQ": nc.dram_tensor("WQ", [128, KT, 256], BF16, kind="ExternalInput").ap(),
        "WK": nc.dram_tensor("WK", [128, KT, 256], BF16, kind="ExternalInput").ap(),
        "WV": nc.dram_tensor("WV", [128, KT, 256], BF16, kind="ExternalInput").ap(),
        "WO": nc.dram_tensor("WO", [128, NPAIRS, D], BF16, kind="ExternalInput").ap(),
        "BQKV": nc.dram_tensor("BQKV", [1, 3, 256], BF16, kind="ExternalInput").ap(),
        "BQKT": nc.dram_tensor("BQKT", [128, 2, NPAIRS], F32, kind="ExternalInput").ap(),
        "OUT": nc.dram_tensor("OUT", [128, NQ, D], BF16, kind="ExternalOutput").ap(),
    }
    with tile.TileContext(nc) as tc:
        _emit(tc, io)
    nc.compile()
    _CACHED = (nc, io)
    return _CACHED


def _shuf_kt(x):
    """[D, N] -> [128, KT, N] with row t*128+p on partition p slot t."""
    n = x.shape[1]
    return np.ascontiguousarray(
        np.asarray(x).reshape(-1, 128, n).transpose(1, 0, 2)).astype(NPBF16)


def make_in_maps(Q, K, V, Wq, bq, Wk, bk, Wv, bv, Wo):
    Q = np.asarray(Q, np.float32)
    K = np.asarray(K, np.float32)
    V = np.asarray(V, np.float32)
    qs = [_shuf_kt(Q[b].T) for b in range(B)]
    ks = [_shuf_kt(K[b].T) for b in range(B)]
    vs = [_shuf_kt(V[b].T) for b in range(B)]
    in_maps = []
    for core in range(NCORES):
        b, g = divmod(core, 4)
        rows = slice(g * 256, (g + 1) * 256)
        wo_g = np.asarray(Wo, np.float32)[:, rows].T  # [256, 1024]
        in_maps.append({
            "QTS": qs[b], "KTS": ks[b], "VTS": vs[b],
            "WQ": _shuf_kt(np.asarray(Wq, np.float32)[rows].T),
            "WK": _shuf_kt(np.asarray(Wk, np.float32)[rows].T),
            "WV": _shuf_kt(np.asarray(Wv, np.float32)[rows].T),
            "WO": np.ascontiguousarray(
                wo_g.reshape(NPAIRS, 128, D).transpose(1, 0, 2)).astype(NPBF16),
            "BQKV": np.stack([
                np.asarray(bq, np.float32)[rows],
                np.asarray(bk, np.float32)[rows],
                np.asarray(bv, np.float32)[rows]])[None].astype(NPBF16),
            "BQKT": np.stack([
                np.asarray(bq, np.float32)[rows].reshape(NPAIRS, 128),
                np.asarray(bk, np.float32)[rows].reshape(NPAIRS, 128),
            ]).transpose(2, 0, 1).astype(np.float32),
        })
    return in_maps


def unshard_out(res_core):
    """[128, NQ, D] core output -> [S, D] float32."""
    r = np.asarray(res_core, np.float32)
    return r.transpose(1, 0, 2).reshape(S, D)


def kernel(Q, K, V, mask, Wq, bq, Wk, bk, Wv, bv, Wo, bo, _results_hook=None):
    nc, _io = _build()
    in_maps = make_in_maps(Q, K, V, Wq, bq, Wk, bk, Wv, bv, Wo)
    res = run_bass_kernel_spmd(nc, in_maps, core_ids=list(range(NCORES)))
    if _results_hook is not None:
        _results_hook(res)
    out = np.zeros((B, S, D), np.float32)
    for core in range(NCORES):
        out[core // 4] += unshard_out(res.results[core]["OUT"])
    out += np.asarray(bo, np.float32)
    return out


# revision 65
# speedup vs baseline: 1.0181x; 1.0181x over previous
"""Self-contained Trainium2 Bass kernel for MultiHeadAttention (v2).

Problem: B=2, S=2048, D=1024, H=16, hd=64, with the reference's
masked_fill(mask==0, -1e-09) quirk: masked scores become ~0.0, so
exp(masked) == 1.0 in fp32 and every key position participates in the
softmax denominator. Fully-masked key blocks contribute a
block-constant suffix sum of V rows (fs), added via rank-1 matmuls.

Sharding: 8 cores = 2 batches x 4 head-groups (4 heads per core).
Each core computes a partial [S, D] output (its 4 heads pushed through
the O-projection); the host sums the 4 partials per batch and adds bo.

v2 scheduling vs v1:
 - host pre-shuffles all DRAM tensors into SBUF layout so every load is
   a large simple descriptor; ~6 prioritized descriptors per DGE queue.
 - projection/outproj work woven between attention steps as fillers so
   the PE never idles while ACT runs exp (PE p-state stays high).
 - diag-first kj order per chunk; last attnU of each chunk is non-diag.
 - finalize: aups -> SBUF copy (frees PSUM fast), DVE reciprocal +
   GpSimd partition_broadcast instead of ACT Ln/Exp + DRAM round-trip.
 - folded suffixes on GpSimd (15 four-head adds), off the DVE.
 - per-st [128, 1024] output tiles, one DMA per st (16 total).
"""

import numpy as np
import ml_dtypes

import concourse.bass as bass
import concourse.bacc as bacc
import concourse.tile as tile
import concourse.mybir as mybir
from concourse.bass_utils import run_bass_kernel_spmd

BF16 = mybir.dt.bfloat16
F32 = mybir.dt.float32
NPBF16 = ml_dtypes.bfloat16
AF = mybir.ActivationFunctionType

B = 2
S = 2048
D = 1024
H = 16
HD = 64
NCORES = 8
HPC = 4            # heads per core
NPAIRS = 2         # head pairs per core
NQ = S // 128      # 16 query/key blocks of 128
QCH = 512          # query chunk width
NCH = S // QCH     # 4 chunks
KT = D // 128      # 8 contraction tiles for projections


def _emit(tc: tile.TileContext, io: dict):
    nc = tc.nc

    persist = tc.alloc_tile_pool(name="persist", bufs=1)

    # ---- constants (gpsimd; must precede first use) ----
    ones128 = persist.tile([128, 128], BF16, name="ones128")
    nc.gpsimd.memset(ones128, 1.0)

    # ---- persistent SBUF arrays ----
    qt = persist.tile([128, NPAIRS, S], BF16, name="qt")
    ktz = persist.tile([128, HPC, S], BF16, name="ktz")
    v2 = persist.tile([128, HPC, NQ, 65], BF16, name="v2")
    fs = persist.tile([128, HPC, NQ, 65], BF16, name="fs")
    att = persist.tile([128, NPAIRS, S], BF16, name="att")

    qts = persist.tile([128, KT, S], BF16, name="qts")
    kts = persist.tile([128, KT, S], BF16, name="kts")
    vts = persist.tile([128, KT, S], BF16, name="vts")
    wq = persist.tile([128, KT, 256], BF16, name="wq")
    wk = persist.tile([128, KT, 256], BF16, name="wk")
    wv = persist.tile([128, KT, 256], BF16, name="wv")
    wo = persist.tile([128, NPAIRS, D], BF16, name="wo")
    bqkv = persist.tile([1, 3, 256], BF16, name="bqkv")
    bqkt = persist.tile([128, 2, NPAIRS], F32, name="bqkt")
    bvb = persist.tile([128, 256], BF16, name="bvb")

    # ---- input DMA: few big descriptors, priority-ordered per queue ----
    # HWDGE queues: sync (SP) + scalar (Activation); gpsimd uses SWDGE.
    # Ordering matches the schedule's consumption: K+wk first (kproj(0)
    # opens the kernel), V chunk0 by ~20us, Q/K chunk1 by ~32us (qp1/kp1),
    # the rest of V by ~45us (fs suffix chain), Q/K chunks 2-3 by ~55us.
    sy, gp, sc = nc.sync, nc.gpsimd, nc.scalar
    # P0 (kq-proj inputs) + ALL of V balanced at 2.5MB per queue: the fs
    # suffix chain (-> finalize(0)) is the startup critical path and needs
    # v2 complete; vts lands ~38us this way.
    sc.dma_start(bqkv[:, :, :], io["BQKV"][:, :, :])
    sc.dma_start(bqkt[:, :, :], io["BQKT"][:, :, :])
    sy.dma_start(wk[:, :, :], io["WK"][:, :, :])
    gp.dma_start(wq[:, :, :], io["WQ"][:, :, :])
    sy.dma_start(kts[:, 0:4, 0:QCH], io["KTS"][:, 0:4, 0:QCH])
    sc.dma_start(kts[:, 4:8, 0:QCH], io["KTS"][:, 4:8, 0:QCH])
    gp.dma_start(qts[:, 0:4, 0:QCH], io["QTS"][:, 0:4, 0:QCH])
    sc.dma_start(qts[:, 4:8, 0:QCH], io["QTS"][:, 4:8, 0:QCH])
    # ktz zero halves + v2 ones column + fs seed (gpsimd, between its DMAs)
    for h in range(HPC):
        half = slice(64, 128) if h % 2 == 0 else slice(0, 64)
        nc.gpsimd.memset(ktz[half, h, :], 0.0)
    nc.gpsimd.memset(v2[:, :, :, 64:65], 1.0)
    nc.gpsimd.memset(fs[:, :, NQ - 1, :], 0.0)
    nc.gpsimd.partition_broadcast(bvb[:, :], bqkv[0:1, 2, :], channels=128)
    sy.dma_start(vts[:, 0:4, 0:QCH], io["VTS"][:, 0:4, 0:QCH])
    gp.dma_start(vts[:, 4:8, 0:QCH], io["VTS"][:, 4:8, 0:QCH])
    sc.dma_start(wv[:, :, :], io["WV"][:, :, :])
    # V-rest DESCENDING (vp15..vp4 drain order: the fs suffix chain runs
    # descending, so its adds pipeline right behind the vproj pipeline)
    sy.dma_start(vts[:, 0:4, 3 * QCH:], io["VTS"][:, 0:4, 3 * QCH:])
    gp.dma_start(vts[:, 4:8, 3 * QCH:], io["VTS"][:, 4:8, 3 * QCH:])
    sy.dma_start(vts[:, 0:4, 2 * QCH:3 * QCH], io["VTS"][:, 0:4, 2 * QCH:3 * QCH])
    gp.dma_start(vts[:, 4:8, 2 * QCH:3 * QCH], io["VTS"][:, 4:8, 2 * QCH:3 * QCH])
    sy.dma_start(vts[:, 0:4, QCH:2 * QCH], io["VTS"][:, 0:4, QCH:2 * QCH])
    gp.dma_start(vts[:, 4:8, QCH:2 * QCH], io["VTS"][:, 4:8, QCH:2 * QCH])
    # Q/K chunk1 (qp1 at end of chunk0, kp1 early chunk1), wo
    sc.dma_start(wo[:, :, :], io["WO"][:, :, :])
    sy.dma_start(qts[:, :, QCH:2 * QCH], io["QTS"][:, :, QCH:2 * QCH])
    gp.dma_start(kts[:, :, QCH:2 * QCH], io["KTS"][:, :, QCH:2 * QCH])
    # Q/K chunks 2+3
    sy.dma_start(qts[:, :, 2 * QCH:3 * QCH], io["QTS"][:, :, 2 * QCH:3 * QCH])
    gp.dma_start(kts[:, :, 2 * QCH:3 * QCH], io["KTS"][:, :, 2 * QCH:3 * QCH])
    sy.dma_start(qts[:, :, 3 * QCH:], io["QTS"][:, :, 3 * QCH:])
    gp.dma_start(kts[:, :, 3 * QCH:], io["KTS"][:, :, 3 * QCH:])

    # ---- pools ----
    pb_s = tc.alloc_tile_pool(name="pb_s", bufs=2, space="PSUM")   # 4 banks
    pb_a = tc.alloc_tile_pool(name="pb_a", bufs=2, space="PSUM")   # 4 banks
    pb_e = tc.alloc_tile_pool(name="pb_e", bufs=10)
    pb_r = tc.alloc_tile_pool(name="pb_r", bufs=2)
    pb_o = tc.alloc_tile_pool(name="pb_o", bufs=4)

    # ---- work units ----
    def qproj_unit(c, p):
        sq = slice(c * QCH, (c + 1) * QCH)
        ps = pb_s.tile([128, 2, QCH], F32, tag="sps", name=f"psq{c}_{p}")
        for t in range(KT):
            nc.tensor.matmul(ps[:, 0, :], wq[:, t, p * 128:(p + 1) * 128],
                             qts[:, t, sq], start=(t == 0), stop=(t == KT - 1))
        nc.vector.tensor_add(qt[:, p, sq], ps[:, 0, :],
                             bqkt[:, 0, p].broadcast_to([128, QCH]))

    def kproj_unit(c, p):
        sq = slice(c * QCH, (c + 1) * QCH)
        ps = pb_s.tile([128, 2, QCH], F32, tag="sps", name=f"psk{c}_{p}")
        for t in range(KT):
            nc.tensor.matmul(ps[:, 0, :], wk[:, t, p * 128:(p + 1) * 128],
                             kts[:, t, sq], start=(t == 0), stop=(t == KT - 1))
        nc.vector.tensor_add(ktz[0:64, 2 * p, sq], ps[0:64, 0, :],
                             bqkt[0:64, 1, p].broadcast_to([64, QCH]))
        nc.vector.tensor_add(ktz[64:128, 2 * p + 1, sq], ps[64:128, 0, :],
                             bqkt[64:128, 1, p].broadcast_to([64, QCH]))

    def vproj_unit(st):
        ps = pb_s.tile([128, 2, QCH], F32, tag="sps", name=f"psv{st}")
        for t in range(KT):
            nc.tensor.matmul(ps[:, 0, 0:256], vts[:, t, st * 128:(st + 1) * 128],
                             wv[:, t, :], start=(t == 0), stop=(t == KT - 1))
        nc.vector.tensor_add(v2[:, :, st, 0:64], ps[:, 0, 0:256], bvb)

    def fs_add(q):
        # fs[:, :, q] = fs[:, :, q+1] + v2[:, :, q+1]   (all 4 heads)
        nc.vector.tensor_add(fs[:, :, q, :], fs[:, :, q + 1, :],
                             v2[:, :, q + 1, :])

    ob_tiles = {}

    def outproj_unit(st, dc):
        ps = pb_s.tile([128, 2, QCH], F32, tag="sps", name=f"pso{st}_{dc}")
        for p in range(NPAIRS):
            nc.tensor.matmul(ps[:, 0, :], att[:, p, st * 128:(st + 1) * 128],
                             wo[:, p, dc * QCH:(dc + 1) * QCH],
                             start=(p == 0), stop=(p == NPAIRS - 1))
        if dc == 0:
            ob_tiles[st] = pb_o.tile([128, D], BF16, tag="ob", name=f"ob{st}")
        ob = ob_tiles[st]
        nc.vector.tensor_copy(ob[:, dc * QCH:(dc + 1) * QCH], ps[:, 0, :])
        if st >= 12:  # tail: half-size DMAs, each half leaves ASAP
            eng = (sy, gp, sc)[(2 * st + dc) % 3]
            eng.dma_start(io["OUT"][:, st, dc * QCH:(dc + 1) * QCH],
                          ob[:, dc * QCH:(dc + 1) * QCH])
        elif dc == 1:
            # sync only: scalar/gpsimd issues would stall ACT exps and
            # affine_selects mid-kernel
            sy.dma_start(io["OUT"][:, st, :], ob)

    # ---- attention ----
    aups_tiles = {}

    def attn_score(c, p, kj):
        c0 = max(kj - 4 * c, 0) * 128
        sps = pb_s.tile([128, 2, QCH], F32, tag="sps", name=f"sps{c}_{p}_{kj}")
        for hl in range(2):
            nc.tensor.matmul(
                sps[:, hl, c0:QCH],
                ktz[:, 2 * p + hl, kj * 128:(kj + 1) * 128],
                qt[:, p, c * QCH + c0:(c + 1) * QCH],
                start=True, stop=True)
        ext = pb_e.tile([128, 2, QCH], BF16, tag="ext", name=f"ext{c}_{p}_{kj}")
        nc.scalar.activation(ext[:, :, c0:QCH], sps[:, :, c0:QCH],
                             AF.Exp, scale=0.125)
        if kj >= 4 * c:  # diagonal block: masked exp entries -> 1.0
            for hl in range(2):
                nc.gpsimd.affine_select(
                    out=ext[:, hl, c0:c0 + 128],
                    in_=ext[:, hl, c0:c0 + 128],
                    compare_op=mybir.AluOpType.is_ge,
                    fill=1.0, base=0,
                    pattern=[[1, 128]], channel_multiplier=-1)
        return ext

    def attn_acc(c, p, kj, ext, first):
        c0 = max(kj - 4 * c, 0) * 128
        aups = aups_tiles[(c, p)]
        for hl in range(2):
            nc.tensor.matmul(aups[:, hl, c0:QCH],
                             v2[:, 2 * p + hl, kj, :],
                             ext[:, hl, c0:QCH],
                             start=first, stop=False)

    def finalize(c, per_st=False):
        ch = slice(c * QCH, (c + 1) * QCH)
        for p in range(NPAIRS):
            aups = aups_tiles[(c, p)]
            for hl in range(2):
                qls = [ql for ql in range(4) if 4 * c + ql < NQ - 1]
                for ql in qls:
                    nc.tensor.matmul(
                        aups[:, hl, ql * 128:(ql + 1) * 128],
                        fs[:, 2 * p + hl, 4 * c + ql, :], ones128,
                        start=False, stop=(ql == qls[-1]))
        # per-pair reciprocal chains, reading aups (PSUM) directly:
        # denom row -> partition 0 (DVE shifted copy), exp(-ln(d)) on ACT,
        # broadcast on GpSimd, normalize on DVE
        d0s, reps = [], []
        for p in range(NPAIRS):
            d0 = pb_r.tile([1, 2, QCH], F32, tag="d0", name=f"d0{c}_{p}")
            nc.vector.tensor_copy(d0[0:1, :, :],
                                  aups_tiles[(c, p)][64:65, :, :])
            d0s.append(d0)
        for p in range(NPAIRS):
            d0 = d0s[p]
            nc.scalar.activation(d0[0:1, :, :], d0[0:1, :, :], AF.Ln)
            nc.scalar.activation(d0[0:1, :, :], d0[0:1, :, :], AF.Exp,
                                 scale=-1.0)
            rep = pb_r.tile([64, 2, QCH], F32, tag="rep", name=f"rep{c}_{p}")
            nc.gpsimd.partition_broadcast(rep[0:64, :, :], d0[0:1, :, :],
                                          channels=64)
            reps.append(rep)
        if per_st:  # tail chunk: st-granular muls so outproj starts early
            for ql in range(4):
                qs = slice(ql * 128, (ql + 1) * 128)
                for p in range(NPAIRS):
                    for hl in range(2):
                        nc.vector.tensor_mul(
                            att[hl * 64:(hl + 1) * 64, p,
                                c * QCH + ql * 128:c * QCH + (ql + 1) * 128],
                            aups_tiles[(c, p)][0:64, hl, qs],
                            reps[p][0:64, hl, qs])
        else:
            for p in range(NPAIRS):
                for hl in range(2):
                    nc.vector.tensor_mul(
                        att[hl * 64:(hl + 1) * 64, p, ch],
                        aups_tiles[(c, p)][0:64, hl, :],
                        reps[p][0:64, hl, :])

    # ---- schedule ----
    kproj_unit(0, 0)
    qproj_unit(0, 0)
    kproj_unit(0, 1)
    qproj_unit(0, 1)

    def chunk(c, kj_order, fillers, per_step):
        """Attention steps with the attnU of step i emitted during step i+1
        (1-step software pipeline: PE never sits on the exp latency)."""
        for p in range(NPAIRS):
            aups_tiles[(c, p)] = pb_a.tile([65, 2, QCH], F32, tag="aups",
                                           name=f"aups{c}_{p}")
        prev = None
        for i, kj in enumerate(kj_order):
            exts = [attn_score(c, p, kj) for p in range(NPAIRS)]
            if prev is not None:
                pkj, pexts, pfirst = prev
                for p in range(NPAIRS):
                    attn_acc(c, p, pkj, pexts[p], first=pfirst)
            prev = (kj, exts, i == 0)
            n = per_step[i] if i < len(per_step) else 0
            for _ in range(n):
                if fillers:
                    fillers.pop(0)()
        pkj, pexts, pfirst = prev
        for p in range(NPAIRS):
            attn_acc(c, p, pkj, pexts[p], first=pfirst)
        while fillers:
            fillers.pop(0)()

    def fs_all():
        for q in range(NQ - 2, -1, -1):
            fs_add(q)

    # chunk 0 fillers: all remaining vproj (V is DMA-prioritized), the fs
    # chain behind them, then qp1 as its DMA lands; finalize(0) right after
    # chunk 0: weave only vp0..3 (needed by the accs); everything else
    # (vp4..15 DMA-paced, fs chain, qp1) drains AFTER the last acc so
    # finalize(0) is reached as soon as the fs data allows.
    fl0 = [lambda st=st: vproj_unit(st) for st in range(4)]
    fl0 += [lambda: vproj_unit(NQ - 1)]
    for st in range(NQ - 2, 3, -1):
        def unit(st=st):
            vproj_unit(st)
            fs_add(st)
        fl0.append(unit)
    fl0 += [lambda q=q: fs_add(q) for q in (3, 2, 1, 0)]
    chunk(0, [0, 1, 2, 3], fl0, [1, 1, 1, 1])
    finalize(0)
    # qk1 during finalize(0)'s ACT/GpSimd chain (PE is free there)
    qproj_unit(1, 0)
    qproj_unit(1, 1)
    kproj_unit(1, 0)
    kproj_unit(1, 1)

    # chunk 1: completely clean ACT-bound steps
    chunk(1, [0, 1, 2, 3, 4, 5, 6, 7], [], [0] * 8)
    finalize(1)
    # qk2 + outproj(0) densely at the boundary, overlapping fin1's chain
    qproj_unit(2, 0)
    qproj_unit(2, 1)
    kproj_unit(2, 0)
    kproj_unit(2, 1)
    for st in range(0, 4):
        for dc in range(2):
            outproj_unit(st, dc)

    # chunk 2: clean steps (kp2 ran at the end of chunk 1)
    chunk(2, [0, 1, 8, 9, 10, 11, 2, 3, 4, 5, 6, 7], [], [0] * 12)
    finalize(2)
    qproj_unit(3, 0)
    qproj_unit(3, 1)
    kproj_unit(3, 0)
    kproj_unit(3, 1)

    # chunk 3: five score-steps pre-run (deep ext pool), then the dense
    # outproj(1)+outproj(2) block executes on PE while ACT drains their
    # exps; remaining steps are ACT-bound with a warm pipeline.
    fl3 = [lambda st=st, dc=dc: outproj_unit(st, dc)
           for st in range(4, 12) for dc in range(2)]
    chunk(3, [0, 1, 12, 13, 14, 15, 2, 3, 4, 5, 6, 7, 8, 9, 10, 11], fl3,
          [0, 0, 0, 0, 16, 0, 0, 0, 0, 0, 0, 0, 0, 0, 0, 0])
    finalize(3, per_st=True)
    for st in range(12, 16):
        for dc in range(2):
            outproj_unit(st, dc)

    pb_o.release()
    pb_r.release()
    pb_e.release()
    pb_a.release()
    pb_s.release()
    persist.release()


_CACHED = None


def _patch_act_tables():
    """Make Exp and Ln resolve to the single combined table set so the
    per-chunk recip (Ln/Exp) doesn't thrash ACT_TABLE_LOADs against the
    softmax Exp calls. Set positions (= act_func_set_id) are preserved;
    only membership of Exp/Ln in other sets is hidden from the selector."""
    from concourse import hw_specs
    orig = hw_specs.get_activation_tables

    def patched(arch):
        t = dict(orig(arch))
        if "natural_log_exp_and_others" in t:
            for name in t:
                if name != "natural_log_exp_and_others":
                    t[name] = t[name] - {AF.Exp, AF.Ln}
        return t

    bacc.get_activation_tables = patched


def _build():
    global _CACHED
    if _CACHED is not None:
        return _CACHED
    _patch_act_tables()
    nc = bacc.Bacc("TRN2", target_bir_lowering=False, debug=False)
    io = {
        "QTS": nc.dram_tensor("QTS", [128, KT, S], BF16, kind="ExternalInput").ap(),
        "KTS": nc.dram_tensor("KTS", [128, KT, S], BF16, kind="ExternalInput").ap(),
        "VTS": nc.dram_tensor("VTS", [128, KT, S], BF16, kind="ExternalInput").ap(),
        "WQ": nc.dram_tensor("WQ", [128, KT, 256], BF16, kind="ExternalInput").ap(),
        "WK": nc.dram_tensor("WK", [128, KT, 256], BF16, kind="ExternalInput").ap(),
        "WV": nc.dram_tensor("WV", [128, KT, 256], BF16, kind="ExternalInput").ap(),
        "WO": nc.dram_tensor("WO", [128, NPAIRS, D], BF16, kind="ExternalInput").ap(),
        "BQKV": nc.dram_tensor("BQKV", [1, 3, 256], BF16, kind="ExternalInput").ap(),
        "BQKT": nc.dram_tensor("BQKT", [128, 2, NPAIRS], F32, kind="ExternalInput").ap(),
        "OUT": nc.dram_tensor("OUT", [128, NQ, D], BF16, kind="ExternalOutput").ap(),
    }
    with tile.TileContext(nc) as tc:
        _emit(tc, io)
    nc.compile()
    _CACHED = (nc, io)
    return _CACHED


def _shuf_kt(x):
    """[D, N] -> [128, KT, N] with row t*128+p on partition p slot t."""
    n = x.shape[1]
    return np.ascontiguousarray(
        np.asarray(x).reshape(-1, 128, n).transpose(1, 0, 2)).astype(NPBF16)


def make_in_maps(Q, K, V, Wq, bq, Wk, bk, Wv, bv, Wo):
    Q = np.asarray(Q, np.float32)
    K = np.asarray(K, np.float32)
    V = np.asarray(V, np.float32)
    qs = [_shuf_kt(Q[b].T) for b in range(B)]
    ks = [_shuf_kt(K[b].T) for b in range(B)]
    vs = [_shuf_kt(V[b].T) for b in range(B)]
    in_maps = []
    for core in range(NCORES):
        b, g = divmod(core, 4)
        rows = slice(g * 256, (g + 1) * 256)
        wo_g = np.asarray(Wo, np.float32)[:, rows].T  # [256, 1024]
        in_maps.append({
            "QTS": qs[b], "KTS": ks[b], "VTS": vs[b],
            "WQ": _shuf_kt(np.asarray(Wq, np.float32)[rows].T),
            "WK": _shuf_kt(np.asarray(Wk, np.float32)[rows].T),
            "WV": _shuf_kt(np.asarray(Wv, np.float32)[rows].T),
            "WO": np.ascontiguousarray(
                wo_g.reshape(NPAIRS, 128, D).transpose(1, 0, 2)).astype(NPBF16),
            "BQKV": np.stack([
                np.asarray(bq, np.float32)[rows],
                np.asarray(bk, np.float32)[rows],
                np.asarray(bv, np.float32)[rows]])[None].astype(NPBF16),
            "BQKT": np.stack([
                np.asarray(bq, np.float32)[rows].reshape(NPAIRS, 128),
                np.asarray(bk, np.float32)[rows].reshape(NPAIRS, 128),
            ]).transpose(2, 0, 1).astype(np.float32),
        })
    return in_maps


def unshard_out(res_core):
    """[128, NQ, D] core output -> [S, D] float32."""
    r = np.asarray(res_core, np.float32)
    return r.transpose(1, 0, 2).reshape(S, D)


def kernel(Q, K, V, mask, Wq, bq, Wk, bk, Wv, bv, Wo, bo, _results_hook=None):
    nc, _io = _build()
    in_maps = make_in_maps(Q, K, V, Wq, bq, Wk, bk, Wv, bv, Wo)
    res = run_bass_kernel_spmd(nc, in_maps, core_ids=list(range(NCORES)))
    if _results_hook is not None:
        _results_hook(res)
    out = np.zeros((B, S, D), np.float32)
    for core in range(NCORES):
        out[core // 4] += unshard_out(res.results[core]["OUT"])
    out += np.asarray(bo, np.float32)
    return out
